# revision 5
# baseline (speedup 1.0000x reference)
"""Trainium2 Bass kernel v3.1 for 3-layer GAT + BN/ELU
(nn_GAT_BN_60859686584881).

Design:
- lane = destination node; per-edge source rows fetched with dma_gather
  (1024 rows / 512B each per instruction — the SWDGE ring limit; larger
  gathers hang the ucode). Softmax accumulation is a PSUM identity-matmul.
- halves A/B are SLOT RANGES (A = slots 0-23 of every core -> table rows
  [0,24576), B = slots 24-48 -> [0,25600) in a second table), keeping both
  gather index spaces within int16. A greedy balancer assigns each node's
  half to equalize every dst's A/B in-edge counts, which tightens the
  per-slot max chunk counts (gather padding 37% -> 21%).
- each layer: PASS-A (A-source chunks for all slots, stashed to SBUF),
  then PASS-B (B-source chunks + finalize). Collectives block their
  issuing (Pool) queue, so AG_B(l) sits between PASS-A(l) and PASS-B(l)
  (hidden by the PASS-A compute tail) and AG_A(l+1) at layer end (hidden
  by the finalize/out-DMA tail).
- leaky-relu as one scalar_tensor_tensor; exp + copies on the Act engine;
  BN scale folded into W columns host-side (shift-only BN add on device);
  ELU recomposed as max(y,0)-0.2 + 0.2*exp(min(y,0)).
"""
import sys
sys.path.insert(0, '/opt/trn_rl_repo')
import numpy as np
import ml_dtypes

import concourse.bacc as bacc
import concourse.bass as bass
import concourse.tile as tile
import concourse.mybir as mybir
from concourse import bass_utils
from concourse.masks import make_identity

N = 50000
E = 800000
F_IN, HID, H, LBL = 512, 16, 8, 40
HC = 128
BN_EPS = 1e-5
P = 128
NCORE = 8
NSLOT = 49
SA = 24                   # A slots per core
SB = NSLOT - SA           # 25
NA = NCORE * SA * P       # 24576 rows in half A
NB = NCORE * SB * P       # 25600 rows in half B
RPC = NSLOT * P           # 6272 rows per core
NROW = NA + NB            # 50176
NDUM = NROW - N           # 176
QUAD = 4
GCAP = 8                  # max chunks (x128 idxs) per dma_gather
GRP = 8                   # chunks per vector-op group

f32 = mybir.dt.float32
bf16 = mybir.dt.bfloat16
i16 = mybir.dt.int16
BFNP = ml_dtypes.bfloat16
Act = mybir.ActivationFunctionType
Alu = mybir.AluOpType

_CACHE = {}


def _grow(c, s, lane):
    """global table row for (core, slot, lane)"""
    return np.where(s < SA, c * (SA * P) + s * P + lane,
                    NA + c * (SB * P) + (s - SA) * P + lane)


# --------------------------------------------------------------- host prep
def _preprocess(edge_index):
    src = np.asarray(edge_index[0], np.int64)
    dst = np.asarray(edge_index[1], np.int64)
    loops = np.arange(N, dtype=np.int64)
    src = np.concatenate([src, loops])
    dst = np.concatenate([dst, loops])

    deg = np.bincount(dst, minlength=N)

    nA_real = NA - NDUM
    order0 = np.argsort(deg, kind='stable')
    Hh = np.zeros(N, np.int8)
    picks = np.floor(np.arange(1, N + 1) * nA_real / N).astype(np.int64)
    isA = np.diff(np.concatenate([[0], picks])) > 0
    Hh[order0[~isA]] = 1

    # greedy balance: pick each node's half to equalize every dst's A/B
    # in-edge counts (shrinks per-slot max chunk counts -> less gather pad)
    so = np.argsort(src, kind='stable')
    dst_by_src = dst[so]
    sptr = np.searchsorted(src[so], np.arange(N + 1))
    eB0 = np.zeros(N, np.int64)
    np.add.at(eB0, dst, Hh[src].astype(np.int64))
    x = (deg - eB0) - eB0          # eA - eB per dst
    countA = int((Hh == 0).sum())
    rng = np.random.default_rng(12345)
    for _sweep in range(4):
        moved = 0
        for n in rng.permutation(N):
            ds = dst_by_src[sptr[n]:sptr[n + 1]]
            S = int(x[ds].sum())
            k = len(ds)
            if Hh[n] == 0:
                if S > k and countA - 1 >= nA_real:
                    Hh[n] = 1
                    np.subtract.at(x, ds, 2)
                    countA -= 1
                    moved += 1
            else:
                if -S > k and countA + 1 <= NA:
                    Hh[n] = 0
                    np.add.at(x, ds, 2)
                    countA += 1
                    moved += 1
        if moved == 0:
            break

    eB = np.zeros(N, np.int64)
    np.add.at(eB, dst, Hh[src].astype(np.int64))
    eA = deg - eB

    snake = np.where(deg % 2 == 0, eA, 50000 - eA)
    key = deg * 100000 + snake
    A_nodes = np.where(Hh == 0)[0]
    A_sorted = A_nodes[np.argsort(key[A_nodes], kind='stable')]
    B_nodes = np.where(Hh == 1)[0]
    B_sorted = B_nodes[np.argsort(key[B_nodes], kind='stable')]

    trow = np.empty(N, np.int64)
    row2node = np.full(NROW, -1, np.int64)
    ndumA = NA - len(A_sorted)
    ndumB = NB - len(B_sorted)
    assert ndumA >= 0 and ndumB >= 0 and ndumA + ndumB == NDUM
    # fill half A: slot-major, within slot core-major then lane; dummies first
    ai = -ndumA
    for s in range(SA):
        for c in range(NCORE):
            base = c * (SA * P) + s * P
            for lane in range(P):
                if ai >= 0:
                    trow[A_sorted[ai]] = base + lane
                    row2node[base + lane] = A_sorted[ai]
                ai += 1
    bi = -ndumB
    for s in range(SA, NSLOT):
        for c in range(NCORE):
            base = NA + c * (SB * P) + (s - SA) * P
            for lane in range(P):
                if bi >= 0:
                    trow[B_sorted[bi]] = base + lane
                    row2node[base + lane] = B_sorted[bi]
                bi += 1
    assert ai == len(A_sorted) and bi == len(B_sorted), (ai, bi)

    # per-row eA/eB and per-slot chunk counts (max over the 1024 rows of the
    # slot across all cores)
    eA_row = np.zeros(NROW, np.int64)
    eB_row = np.zeros(NROW, np.int64)
    m = row2node >= 0
    eA_row[m] = eA[row2node[m]]
    eB_row[m] = eB[row2node[m]]
    # slot of a global row
    rows_all = np.arange(NROW)
    in_a = rows_all < NA
    s_of = np.where(in_a, (rows_all % (SA * P)) // P,
                    SA + ((rows_all - NA) % (SB * P)) // P)
    core_of = np.where(in_a, rows_all // (SA * P), (rows_all - NA) // (SB * P))
    lane_of = rows_all % P
    nchA = np.zeros(NSLOT, np.int64)
    nchB = np.zeros(NSLOT, np.int64)
    for s in range(NSLOT):
        msk = s_of == s
        nchA[s] = eA_row[msk].max()
        nchB[s] = eB_row[msk].max()

    # quad-major column layout: per quad: A cols (slots in order), B cols
    col_off_A = np.zeros(NSLOT, np.int64)
    col_off_B = np.zeros(NSLOT, np.int64)
    off = 0
    for q0 in range(0, NSLOT, QUAD):
        qs = range(q0, min(q0 + QUAD, NSLOT))
        for s in qs:
            col_off_A[s] = off
            off += nchA[s]
        for s in qs:
            col_off_B[s] = off
            off += nchB[s]
    TOT = int(off)

    # per-edge slot fill
    dst_row = trow[dst]
    src_row = trow[src]
    src_half = (src_row >= NA).astype(np.int8)
    ordere = np.lexsort((src_half, dst_row))
    dst_row_s = dst_row[ordere]
    src_row_s = src_row[ordere]
    half_s = src_half[ordere]
    cnts = np.bincount(dst_row_s, minlength=NROW)
    starts = np.concatenate([[0], np.cumsum(cnts)])

    idx_flat = np.zeros((NCORE, TOT * P), np.int16)   # 0 = pad
    e_core = core_of[dst_row_s]
    e_slot = s_of[dst_row_s]
    e_lane = lane_of[dst_row_s]
    idx_in_run = np.arange(len(dst_row_s)) - starts[dst_row_s]
    nA_of_dst = eA_row[dst_row_s]
    is_b = half_s == 1
    j = np.where(is_b, idx_in_run - nA_of_dst, idx_in_run)
    col = np.where(is_b, col_off_B[e_slot] + j, col_off_A[e_slot] + j)
    val = np.where(is_b, 1 + (src_row_s - NA), 1 + src_row_s).astype(np.int16)
    assert val.min() >= 1 and val.max() <= 32767
    idx_flat[e_core, col * P + e_lane] = val

    return dict(trow=trow, row2node=row2node,
                nchA=tuple(int(v) for v in nchA),
                nchB=tuple(int(v) for v in nchB),
                idx_flat=idx_flat, TOT=TOT)


def _wrap16(flat):
    t = flat.reshape(-1, 16).T          # [16, K/16]
    return np.tile(t, (8, 1)).copy()    # [128, K/16]


def _gather_plan(nchA, nchB, gcap=GCAP):
    """offsets per slot + per-quad gather segs per half:
    plan[q] = (qs, {0: [(c0, gn), ...], 1: [...]})"""
    offA, offB = {}, {}
    off = 0
    for q0 in range(0, NSLOT, QUAD):
        qs = list(range(q0, min(q0 + QUAD, NSLOT)))
        for s in qs:
            offA[s] = off
            off += nchA[s]
        for s in qs:
            offB[s] = off
            off += nchB[s]
    plan = []
    for q0 in range(0, NSLOT, QUAD):
        qs = list(range(q0, min(q0 + QUAD, NSLOT)))
        segs = {0: [], 1: []}
        for hf in (0, 1):
            nch = nchA if hf == 0 else nchB
            offs = offA if hf == 0 else offB
            runs = [(offs[s], nch[s]) for s in qs if nch[s] > 0]
            if not runs:
                continue
            start = runs[0][0]
            total = sum(r[1] for r in runs)
            c = start
            while c < start + total:
                gn = min(gcap, start + total - c)
                segs[hf].append((c, gn))
                c += gn
        plan.append((qs, segs))
    return plan, offA, offB


# ------------------------------------------------------------ bass program
def _build_program(nchA, nchB, TOT, skip_ag=False, skip_gather=False,
                   skip_edge=False, skip_compute=False, reps=1,
                   gcap=GCAP, scratch=16384, nqueues=4):
    nc = bacc.Bacc("TRN2", target_bir_lowering=False, debug=False,
                   enable_asserts=False, num_devices=NCORE,
                   num_swdge_queues=nqueues,
                   dynamic_dma_scratch_size=scratch)
    plan, offA, offB = _gather_plan(nchA, nchB, gcap)
    gq = [0]

    xT_t = nc.dram_tensor("xT", [F_IN, RPC], bf16, kind="ExternalInput")
    idx_t = nc.dram_tensor("idx", [P, TOT * 8], i16, kind="ExternalInput")
    w0_t = nc.dram_tensor("w0ext", [F_IN, 144], bf16, kind="ExternalInput")
    w1_t = nc.dram_tensor("w1ext", [HC, 144], f32, kind="ExternalInput")
    w2_t = nc.dram_tensor("w2ext", [HC, 42], f32, kind="ExternalInput")
    bn0_t = nc.dram_tensor("bn0", [P, HC], f32, kind="ExternalInput")
    bn1_t = nc.dram_tensor("bn1", [P, HC], f32, kind="ExternalInput")
    b2_t = nc.dram_tensor("b2", [P, LBL], f32, kind="ExternalInput")
    pad_t = nc.dram_tensor("padrow", [1, 256], bf16, kind="ExternalInput")
    pad2_t = nc.dram_tensor("padrow2", [1, 64], f32, kind="ExternalInput")
    out_t = nc.dram_tensor("out", [RPC, LBL], f32, kind="ExternalOutput")

    NROWH = {0: NA, 1: NB}
    shard = {}
    tabs = {}
    for l in range(3):
        elem = 256 if l < 2 else 64
        dt = bf16 if l < 2 else f32
        for hf, nrows, nloc in ((0, NA, SA * P), (1, NB, SB * P)):
            shard[(l, hf)] = nc.dram_tensor(
                f"shard{l}_{hf}", [nloc, elem], dt, kind="Internal")
            tabs[(l, hf)] = nc.dram_tensor(
                f"tab{l}_{hf}", [1 + nrows, elem], dt, kind="Internal",
                addr_space="Shared")

    def slot_shard_view(l, s):
        """per-core shard rows for slot s of layer-l table"""
        if s < SA:
            return shard[(l, 0)][s * P:(s + 1) * P, :]
        return shard[(l, 1)][(s - SA) * P:(s - SA + 1) * P, :]

    with tile.TileContext(nc) as tc:
        with tc.tile_pool(name="sbuf", bufs=1) as sb, \
             tc.tile_pool(name="psum", bufs=1, space="PSUM") as pp:
          for _rep in range(reps):
            ident = sb.tile([P, P], f32, name="ident")
            make_identity(nc, ident[:])
            ident_bf = sb.tile([P, P], bf16, name="ident_bf")
            nc.gpsimd.tensor_copy(ident_bf[:], ident[:])

            idx_sb = sb.tile([P, TOT * 8], i16, name="idx_sb")
            nc.sync.dma_start(idx_sb[:], idx_t[:])
            w1_sb = sb.tile([HC, 144], f32, name="w1_sb")
            nc.sync.dma_start(w1_sb[:], w1_t[:])
            w2_sb = sb.tile([HC, 42], f32, name="w2_sb")
            nc.sync.dma_start(w2_sb[:], w2_t[:])
            bn_sb = [sb.tile([P, HC], f32, name=f"bn_sb{l}") for l in range(2)]
            nc.sync.dma_start(bn_sb[0][:], bn0_t[:])
            nc.sync.dma_start(bn_sb[1][:], bn1_t[:])
            b2_sb = sb.tile([P, LBL], f32, name="b2_sb")
            nc.sync.dma_start(b2_sb[:], b2_t[:])
            pad_sb = sb.tile([1, 256], bf16, name="pad_sb")
            nc.sync.dma_start(pad_sb[:], pad_t[:])
            pad2_sb = sb.tile([1, 64], f32, name="pad2_sb")
            nc.sync.dma_start(pad2_sb[:], pad2_t[:])
            ad_sb = [sb.tile([P, NSLOT * (H if l < 2 else 1)], f32,
                             name=f"ad_sb{l}") for l in range(3)]
            accA = sb.tile([P, NSLOT * 137], f32, name="accA")

            # pad row 0 of each table
            for l in range(3):
                for hf in (0, 1):
                    psrc = pad_sb if l < 2 else pad2_sb
                    nc.sync.dma_start(tabs[(l, hf)][0:1, :], psrc[:])

            def trigger_ag(l, hf):
                if skip_ag:
                    return
                nc.gpsimd.collective_compute(
                    "AllGather", mybir.AluOpType.bypass,
                    replica_groups=[list(range(NCORE))],
                    ins=[shard[(l, hf)][:]],
                    outs=[tabs[(l, hf)][1:1 + NROWH[hf], :]])

            # ---------------- prologue: rows0 = xT^T @ w0ext (bf16)
            w0_sb = [sb.tile([P, 144], bf16, name=f"w0_sb{k}") for k in range(4)]
            for k in range(4):
                nc.sync.dma_start(w0_sb[k][:], w0_t[k * P:(k + 1) * P, :])
            xT_sb = [sb.tile([P, RPC], bf16, name=f"xT_sb{k}") for k in range(4)]
            for k in range(4):
                nc.sync.dma_start(xT_sb[k][:], xT_t[k * P:(k + 1) * P, :])
            for s in range(NSLOT):
                ps = pp.tile([P, 144], f32, name="ps_pro", tag="misc_ps", bufs=2)
                for k in range(4):
                    nc.tensor.matmul(out=ps[:],
                                     lhsT=xT_sb[k][:, s * P:(s + 1) * P],
                                     rhs=w0_sb[k][:],
                                     start=(k == 0), stop=(k == 3))
                row_bf = sb.tile([P, 256], bf16, name="row_pro", tag="row_pro",
                                 bufs=2)
                nc.scalar.activation(row_bf[:, :136], ps[:, :136], Act.Copy)
                nc.scalar.activation(ad_sb[0][:, s * H:(s + 1) * H],
                                     ps[:, 136:144], Act.Copy)
                nc.sync.dma_start(slot_shard_view(0, s), row_bf[:])
            trigger_ag(0, 0)

            # ---------------- edge phases
            for l in range(3):
                nh = H if l < 2 else 1
                hw = HC if l < 2 else LBL
                wm = hw + nh                      # 136 / 136 / 41
                elem = 256 if l < 2 else 64
                mdt = bf16 if l < 2 else f32
                idt = ident_bf if l < 2 else ident
                if skip_edge:
                    continue

                def compute_half(qs, segs, hf, nch_of, off_of, on_done):
                    """gather + compute chunks of one half for a quad.
                    on_done(s, ps_acc_tile_or_None) called per slot after its
                    chunks are accumulated."""
                    gtiles = []
                    for c0, gn in segs:
                        g = sb.tile([P, gn * elem], mdt, name=f"g{l}",
                                    tag="g", bufs=8)
                        view = tabs[(l, hf)][0:1 + NROWH[hf], :]
                        if not skip_gather:
                            nc.gpsimd.dma_gather(
                                out_ap=g[:].rearrange("p (j e) -> p j e", j=gn),
                                in_ap=view,
                                idxs_ap=idx_sb[:, c0 * 8:(c0 + gn) * 8],
                                num_idxs=gn * P, num_idxs_reg=gn * P,
                                elem_size=elem, queue_num=gq[0] % nqueues)
                            gq[0] += 1
                        gtiles.append((c0, gn, g))
                    if skip_compute:
                        for s in qs:
                            on_done(s, None)
                        return
                    for s in qs:
                        nch = nch_of[s]
                        if nch == 0:
                            on_done(s, None)
                            continue
                        acc = pp.tile([P, wm], f32, name=f"acc{l}_{s % QUAD}",
                                      tag=f"acc{s % QUAD}", bufs=1)
                        done = 0
                        cstart = off_of[s]
                        c = cstart
                        while c < cstart + nch:
                            for gc0, ggn, g in gtiles:
                                if gc0 <= c < gc0 + ggn:
                                    break
                            else:
                                raise AssertionError("no seg")
                            gn = min(GRP, cstart + nch - c, gc0 + ggn - c)
                            lc = c - gc0
                            gv = g[:].rearrange("p (j e) -> p j e",
                                                j=ggn)[:, lc:lc + gn, :]
                            u = sb.tile([P, GRP * nh], f32, name=f"u{l}",
                                        tag="u", bufs=4)
                            mt = sb.tile([P, GRP * wm], mdt, name=f"m{l}",
                                         tag="m", bufs=4)
                            uv = u[:, :gn * nh].rearrange(
                                "p (j h) -> p j h", j=gn)
                            nc.vector.tensor_tensor(
                                out=uv,
                                in0=gv[:, :, hw:wm],
                                in1=ad_sb[l][:, s * nh:(s + 1) * nh]
                                    .unsqueeze(1).to_broadcast([P, gn, nh]),
                                op=Alu.add)
                            nc.vector.scalar_tensor_tensor(
                                out=u[:, :gn * nh], in0=u[:, :gn * nh],
                                scalar=0.2, in1=u[:, :gn * nh],
                                op0=Alu.mult, op1=Alu.max)
                            mv = mt[:, :gn * wm].rearrange(
                                "p (j w) -> p j w", j=gn)
                            nc.scalar.activation(
                                mv[:, :, hw:wm], uv, Act.Exp)
                            nc.vector.tensor_tensor(
                                out=mv[:, :, :hw].rearrange(
                                    "p j (h c) -> p j h c", h=nh),
                                in0=gv[:, :, :hw].rearrange(
                                    "p j (h c) -> p j h c", h=nh),
                                in1=mv[:, :, hw:wm].unsqueeze(-1)
                                    .to_broadcast([P, gn, nh, hw // nh]),
                                op=Alu.mult)
                            for jj in range(gn):
                                nc.tensor.matmul(
                                    out=acc[:],
                                    lhsT=idt[:],
                                    rhs=mt[:, jj * wm:(jj + 1) * wm],
                                    start=(done == 0),
                                    stop=(done == nch - 1),
                                    skip_group_check=True)
                                done += 1
                            c += gn
                        assert done == nch
                        on_done(s, acc)

                # -------- PASS A: A-source chunks for every quad
                for qs, segs in plan:
                    def stash(s, acc):
                        if acc is not None:
                            nc.scalar.activation(
                                accA[:, s * 137:s * 137 + wm], acc[:],
                                Act.Copy)
                    compute_half(qs, segs[0], 0, nchA, offA, stash)
                # AG_B(l) sits here: Pool blocks on it while PASS-A compute
                # drains; PASS-B gathers right after need exactly this table
                trigger_ag(l, 1)

                # -------- PASS B: B-source chunks + finalize + AGs
                for qi, (qs, segs) in enumerate(plan):
                    def fin(s, accB):
                        if skip_compute:
                            return
                        comb = sb.tile([P, wm], f32, name=f"comb{l}",
                                       tag="comb", bufs=2)
                        hasA = nchA[s] > 0
                        if accB is not None and hasA:
                            nc.vector.tensor_tensor(
                                out=comb[:], in0=accB[:],
                                in1=accA[:, s * 137:s * 137 + wm], op=Alu.add)
                        elif accB is not None:
                            nc.vector.tensor_copy(comb[:], accB[:])
                        else:
                            nc.vector.tensor_copy(
                                comb[:], accA[:, s * 137:s * 137 + wm])
                        rs = sb.tile([P, nh], f32, name=f"rs{l}", tag="rs",
                                     bufs=2)
                        nc.vector.tensor_scalar_max(rs[:], comb[:, hw:wm],
                                                    1e-30)
                        nc.vector.reciprocal(rs[:], rs[:])
                        ob = sb.tile([P, hw], f32, name=f"ob{l}", tag="ob",
                                     bufs=2)
                        nc.vector.tensor_tensor(
                            out=ob[:].rearrange("p (h c) -> p h c", h=nh),
                            in0=comb[:, :hw].rearrange("p (h c) -> p h c",
                                                       h=nh),
                            in1=rs[:].unsqueeze(-1).to_broadcast(
                                [P, nh, hw // nh]),
                            op=Alu.mult)
                        if l < 2:
                            # + BN shift (scale folded into W cols host-side)
                            nc.vector.tensor_tensor(
                                out=ob[:], in0=ob[:], in1=bn_sb[l][:],
                                op=Alu.add)
                            # ELU: y2=max(ob,0)-0.2 ; t1=min(ob,0);
                            # ob2 = 0.2*exp(t1) + y2
                            tneg = sb.tile([P, hw], f32, name=f"tneg{l}",
                                           tag="tneg", bufs=2)
                            y2 = sb.tile([P, hw], f32, name=f"y2_{l}",
                                         tag="y2", bufs=2)
                            nc.vector.tensor_scalar_min(tneg[:], ob[:], 0.0)
                            nc.vector.tensor_scalar(
                                out=y2[:], in0=ob[:], scalar1=0.0,
                                scalar2=-0.2, op0=Alu.max, op1=Alu.add)
                            nc.scalar.activation(tneg[:], tneg[:], Act.Exp)
                            nc.vector.scalar_tensor_tensor(
                                out=ob[:], in0=tneg[:], scalar=0.2,
                                in1=y2[:], op0=Alu.mult, op1=Alu.add)
                            eT_ps = pp.tile([P, P], f32, name=f"eT{l}",
                                            tag="misc_ps", bufs=2)
                            nc.tensor.transpose(out=eT_ps[:], in_=ob[:],
                                                identity=ident[:])
                            eT_sb = sb.tile([P, P], f32, name=f"eT_sb{l}",
                                            tag="eT_sb", bufs=2)
                            nc.scalar.activation(eT_sb[:], eT_ps[:], Act.Copy)
                            wnext = w1_sb if l == 0 else w2_sb
                            wn = 144 if l == 0 else 42
                            row_ps = pp.tile([P, wn], f32, name=f"rp{l}",
                                             tag="misc_ps", bufs=2)
                            nc.tensor.matmul(out=row_ps[:], lhsT=eT_sb[:],
                                             rhs=wnext[:, :wn],
                                             start=True, stop=True)
                            if l == 0:
                                row_o = sb.tile([P, 256], bf16, name="row1",
                                                tag="row1", bufs=2)
                                nc.scalar.activation(row_o[:, :136],
                                                     row_ps[:, :136], Act.Copy)
                                nc.scalar.activation(
                                    ad_sb[1][:, s * H:(s + 1) * H],
                                    row_ps[:, 136:144], Act.Copy)
                                nc.sync.dma_start(slot_shard_view(1, s),
                                                  row_o[:])
                            else:
                                row_o = sb.tile([P, 64], f32, name="row2",
                                                tag="row2", bufs=2)
                                nc.scalar.activation(row_o[:, :41],
                                                     row_ps[:, :41], Act.Copy)
                                nc.scalar.activation(ad_sb[2][:, s:s + 1],
                                                     row_ps[:, 41:42],
                                                     Act.Copy)
                                nc.sync.dma_start(slot_shard_view(2, s),
                                                  row_o[:])
                        else:
                            nc.vector.tensor_tensor(
                                out=ob[:], in0=ob[:], in1=b2_sb[:],
                                op=Alu.add)
                            nc.sync.dma_start(out_t[s * P:(s + 1) * P, :],
                                              ob[:])

                    compute_half(qs, segs[1], 1, nchB, offB, fin)
                # AG_A(l+1): Pool blocks while the PASS-B finalize tail and
                # out/shard DMAs drain; PASS-A(l+1) needs exactly this table
                if l < 2:
                    trigger_ag(l + 1, 0)
    nc.compile()
    return nc


# ------------------------------------------------------------- host inputs
def _host_inputs(inputs, pp):
    row2node = pp['row2node']
    idx_flat = pp['idx_flat']

    x = np.asarray(inputs['x'], np.float32)
    xr = np.zeros((NROW, F_IN), np.float32)
    m = row2node >= 0
    xr[m] = x[row2node[m]]

    def bd(a):
        hh, cc = a.shape
        mm = np.zeros((hh * cc, hh), np.float32)
        for h in range(hh):
            mm[h * cc:(h + 1) * cc, h] = a[h]
        return mm

    W0, W1, W2 = [np.asarray(inputs[k], np.float32) for k in ('W0', 'W1', 'W2')]

    def bn_fold(g, rv):
        return np.asarray(g) / np.sqrt(np.asarray(rv) + BN_EPS)

    s0 = bn_fold(inputs['g0'], inputs['rv0']).astype(np.float32)
    s1 = bn_fold(inputs['g1'], inputs['rv1']).astype(np.float32)
    w0ext = np.concatenate([W0 * s0[None, :], W0 @ bd(np.asarray(inputs['as0'])),
                            W0 @ bd(np.asarray(inputs['ad0']))], 1)
    w1ext = np.concatenate([W1 * s1[None, :], W1 @ bd(np.asarray(inputs['as1'])),
                            W1 @ bd(np.asarray(inputs['ad1']))], 1)
    w2ext = np.concatenate([W2, W2 @ bd(np.asarray(inputs['as2'])),
                            W2 @ bd(np.asarray(inputs['ad2']))], 1)

    def shift(b, rm, be, s):
        sh = (np.asarray(b, np.float32) - np.asarray(rm, np.float32)) * s \
            + np.asarray(be, np.float32)
        return np.tile(sh[None, :], (P, 1)).astype(np.float32)

    bn0 = shift(inputs['b0'], inputs['rm0'], inputs['be0'], s0)
    bn1 = shift(inputs['b1'], inputs['rm1'], inputs['be1'], s1)
    b2 = np.tile(np.asarray(inputs['b2'], np.float32)[None, :], (P, 1))

    padrow = np.zeros((1, 256), BFNP)
    padrow[0, 128:136] = BFNP(-1e4)
    padrow2 = np.zeros((1, 64), np.float32)
    padrow2[0, 40] = -1e4

    in_maps = []
    for c in range(NCORE):
        loc = np.concatenate([
            xr[c * (SA * P):(c + 1) * (SA * P)],
            xr[NA + c * (SB * P):NA + (c + 1) * (SB * P)]], 0)
        xT = loc.T.astype(BFNP)
        in_maps.append({
            'xT': np.ascontiguousarray(xT),
            'idx': _wrap16(idx_flat[c]),
            'w0ext': w0ext.astype(BFNP), 'w1ext': w1ext, 'w2ext': w2ext,
            'bn0': bn0, 'bn1': bn1, 'b2': b2,
            'padrow': padrow, 'padrow2': padrow2,
        })
    return in_maps


def kernel(**inputs):
    edge_index = np.asarray(inputs['edge_index'])
    pp = _preprocess(edge_index)
    key = (pp['nchA'], pp['nchB'])
    if key not in _CACHE:
        _CACHE[key] = _build_program(pp['nchA'], pp['nchB'], pp['TOT'])
    nc = _CACHE[key]
    in_maps = _host_inputs(inputs, pp)
    res = bass_utils.run_bass_kernel_spmd(nc, in_maps,
                                          core_ids=list(range(NCORE)))
    rows = np.concatenate([res.results[c]['out'] for c in range(NCORE)], 0)
    # local flat row (c*RPC + loc) -> global table row
    c_i = np.arange(NROW) // RPC
    loc = np.arange(NROW) % RPC
    glob = np.where(loc < SA * P, c_i * (SA * P) + loc,
                    NA + c_i * (SB * P) + (loc - SA * P))
    out = np.zeros((N, LBL), np.float32)
    node = pp['row2node'][glob]
    m = node >= 0
    out[node[m]] = rows[m]
    return out
